# revision 6
# baseline (speedup 1.0000x reference)
"""Trainium2 Bass kernel for nn_DynamicsLookAheadModel.

LSTM warm-up over S=96 steps + 32-step look-ahead with output feedback,
data-parallel over the batch (2048) across 8 NeuronCores (256 per core).

Per-core layout (all fp32):
  - Everything "transposed": hidden units on partitions, batch on the free dim.
    H=256 tensors are folded into [128, 512] tiles:
      phys[p, j]       = logical[p,     j]   for j <  256   (h-dims 0..127)
      phys[p, 256 + j] = logical[128+p, j]                  (h-dims 128..255)
  - Gates g = W_ih@x + b + W_hh@h computed on the PE into PSUM, one bank per
    gate, as out = lhsT.T @ rhs with K-tiles {33 (x plus ones-row bias), 128,
    128 (folded h halves)}.
  - Sigmoid/Tanh on ScalarE straight from PSUM; cell update on VectorE.
  - STE binarization uses sign(c') (sigmoid(o) > 0 always, tanh sign-preserving),
    so bits = (c' > 0) via one tensor_scalar is_gt.
  - Outputs are stored per step as [6, 256] and assembled as [33, 6, 256] in
    DRAM; the host gather transposes to [256, 33, 6].
"""

import numpy as np

import concourse.bass as bass
import concourse.mybir as mybir
import concourse.tile as tile
from concourse.bass_utils import run_bass_kernel_spmd

B, S, F, H, O = 2048, 96, 32, 256, 6
LA = 32
NCORES = 8
BL = B // NCORES  # 256 per-core batch
FD = 2 * BL  # 512: folded free dim for H=256 tensors
KX = F + 1  # 33: x features + ones row (bias)
FP32 = mybir.dt.float32


# --- workaround: this walrus build allows only ONE sem wait per instruction ---
# Spill excess semaphore waits onto same-engine NOPs placed just before the
# instruction (engines execute their queue in order, so semantics match).
def _spill_excess_waits(nc, limit=1):
    cnt = 0
    for f in nc.m.functions:
        for bb in f.blocks:
            new_list = []
            for ins in bb.instructions:
                si = ins.sync_info
                if si and si.on_wait and len(si.on_wait) > limit:
                    waits = list(si.on_wait)
                    for w in waits[:-limit]:
                        n = mybir.InstNoOp(name=f"wspill_{cnt}", ins=[], outs=[])
                        cnt += 1
                        n.engine = ins.engine
                        n.sync_info = mybir.SyncInfo(on_wait=[w], on_update=[])
                        new_list.append(n)
                    ins.sync_info = mybir.SyncInfo(
                        on_wait=waits[-limit:], on_update=list(si.on_update)
                    )
                new_list.append(ins)
            bb.instructions[:] = new_list
    return cnt


def build_nc(n_warm=S, n_la=LA, spill=True):
    from contextlib import ExitStack

    nc = bass.Bass()
    AF = mybir.ActivationFunctionType
    ALU = mybir.AluOpType

    xaug_d = nc.dram_tensor("xaug", [n_warm, KX, BL], FP32, kind="ExternalInput")
    wiha_d = nc.dram_tensor("wiha", [KX, 4 * H], FP32, kind="ExternalInput")
    whh0_d = nc.dram_tensor("whh0", [128, 4 * H], FP32, kind="ExternalInput")
    whh1_d = nc.dram_tensor("whh1", [128, 4 * H], FP32, kind="ExternalInput")
    wfc_d = nc.dram_tensor("wfc", [128, 2 * O], FP32, kind="ExternalInput")
    bfc_d = nc.dram_tensor("bfc", [O, 1], FP32, kind="ExternalInput")
    out_d = nc.dram_tensor("out_t", [n_la + 1, O, BL], FP32, kind="ExternalOutput")

    with tile.TileContext(nc) as tc, ExitStack() as es:
        wp_ctx = es.enter_context(tc.tile_pool(name="weights", bufs=1))
        xp_ctx = es.enter_context(tc.tile_pool(name="xtiles", bufs=1))
        sp_ctx = es.enter_context(tc.tile_pool(name="state", bufs=2))
        gp_ctx = es.enter_context(tc.tile_pool(name="gates", bufs=1, space="PSUM"))
        op_ctx = es.enter_context(tc.tile_pool(name="outp", bufs=2, space="PSUM"))

        # weights
        wiha = wp_ctx.tile([KX, 4 * H], FP32, tag="wiha")
        nc.sync.dma_start(out=wiha, in_=wiha_d[:, :])
        whh0 = wp_ctx.tile([128, 4 * H], FP32, tag="whh0")
        nc.sync.dma_start(out=whh0, in_=whh0_d[:, :])
        whh1 = wp_ctx.tile([128, 4 * H], FP32, tag="whh1")
        nc.sync.dma_start(out=whh1, in_=whh1_d[:, :])
        wfc = wp_ctx.tile([128, 2 * O], FP32, tag="wfc")
        nc.sync.dma_start(out=wfc, in_=wfc_d[:, :])
        bfc = wp_ctx.tile([O, 1], FP32, tag="bfc")
        nc.sync.dma_start(out=bfc, in_=bfc_d[:, :])

        # x tiles (one per step; LA reuses tiles 0..31 with rows 0:6 replaced)
        xt = []
        for t in range(n_warm):
            xtile = xp_ctx.tile([KX, BL], FP32, tag=f"x{t}")
            nc.sync.dma_start(out=xtile, in_=xaug_d[t, :, :])
            xt.append(xtile)

        # initial state
        c_prev = sp_ctx.tile([128, FD], FP32, tag="c")
        h_prev = None  # step 0 skips the W_hh matmuls entirely

        # gate order: g first (tanh feeds t2), then f, i, o
        GATES = [("g", 2), ("f", 1), ("i", 0), ("o", 3)]

        def lstm_step(xtile, h_prev, c_prev, first=False):
            ps = {}
            for name, gi in GATES:
                if first and name == "f":
                    continue  # sigmoid(f)*c is 0 at step 0
                p = gp_ctx.tile([128, FD], FP32, tag="p" + name)
                ps[name] = p
                for m in (0, 1):
                    col = gi * H + m * 128
                    osl = p[:, m * BL : (m + 1) * BL]
                    nc.tensor.matmul(
                        osl,
                        wiha[:, col : col + 128],
                        xtile[:, :],
                        start=True,
                        stop=first,
                    )
                    if not first:
                        nc.tensor.matmul(
                            osl,
                            whh0[:, col : col + 128],
                            h_prev[:, 0:BL],
                            start=False,
                            stop=False,
                        )
                        nc.tensor.matmul(
                            osl,
                            whh1[:, col : col + 128],
                            h_prev[:, BL:FD],
                            start=False,
                            stop=True,
                        )

            tg = sp_ctx.tile([128, FD], FP32, tag="tg")
            nc.scalar.activation(out=tg, in_=ps["g"][:, :], func=AF.Tanh)
            if not first:
                sf = sp_ctx.tile([128, FD], FP32, tag="sf")
                nc.scalar.activation(out=sf, in_=ps["f"][:, :], func=AF.Sigmoid)
            si = sp_ctx.tile([128, FD], FP32, tag="si")
            nc.scalar.activation(out=si, in_=ps["i"][:, :], func=AF.Sigmoid)
            so = sp_ctx.tile([128, FD], FP32, tag="so")
            nc.scalar.activation(out=so, in_=ps["o"][:, :], func=AF.Sigmoid)

            c_new = sp_ctx.tile([128, FD], FP32, tag="c")
            if first:
                nc.vector.tensor_tensor(out=c_new, in0=si, in1=tg, op=ALU.mult)
            else:
                t1 = sp_ctx.tile([128, FD], FP32, tag="t1")
                nc.vector.tensor_tensor(out=t1, in0=sf, in1=c_prev, op=ALU.mult)
                t2 = sp_ctx.tile([128, FD], FP32, tag="t2")
                nc.vector.tensor_tensor(out=t2, in0=si, in1=tg, op=ALU.mult)
                nc.vector.tensor_tensor(out=c_new, in0=t1, in1=t2, op=ALU.add)
            tc_t = sp_ctx.tile([128, FD], FP32, tag="tc")
            nc.scalar.activation(out=tc_t, in_=c_new, func=AF.Tanh)
            h_new = sp_ctx.tile([128, FD], FP32, tag="h")
            nc.vector.tensor_tensor(out=h_new, in0=so, in1=tc_t, op=ALU.mult)
            return h_new, c_new

        def emit_output(k, c_cur):
            # bits = (c' > 0); equals STE(h) since sigmoid(o)>0, tanh sign-pres.
            bits = sp_ctx.tile([128, FD], FP32, tag="bits")
            nc.vector.tensor_scalar(
                out=bits, in0=c_cur, scalar1=0.0, scalar2=None, op0=ALU.is_gt
            )
            po = op_ctx.tile([O, BL], FP32, tag="po")
            nc.tensor.matmul(po, wfc[:, 0:O], bits[:, 0:BL], start=True, stop=False)
            nc.tensor.matmul(po, wfc[:, O : 2 * O], bits[:, BL:FD], start=False, stop=True)
            osb = sp_ctx.tile([O, BL], FP32, tag="osb")
            nc.scalar.activation(out=osb, in_=po, func=AF.Identity, bias=bfc)
            nc.sync.dma_start(out=out_d[k, :, :], in_=osb)
            return osb

        # warm-up
        for t in range(n_warm):
            h_prev, c_prev = lstm_step(xt[t], h_prev, c_prev, first=(t == 0))

        # look-ahead: output k uses c' of the step just computed; feed into
        # x tile k (rows 0:6) consumed by LA step k.
        for k in range(n_la + 1):
            osb = emit_output(k, c_prev)
            if k < n_la:
                nc.vector.tensor_copy(out=xt[k][0:O, :], in_=osb)
                h_prev, c_prev = lstm_step(xt[k], h_prev, c_prev)

    if spill:
        _spill_excess_waits(nc)
    return nc


def _host_prep(x, W_ih, W_hh, b_ih, b_hh, W_fc, b_fc):
    """Build the 8 per-core input maps."""
    bias = (b_ih + b_hh).astype(np.float32)
    wiha = np.concatenate([W_ih, bias[:, None]], axis=1).T  # [33, 1024]
    whh_t = np.ascontiguousarray(W_hh.T)  # [256, 1024]
    wfc_fold = np.concatenate([W_fc.T[:128], W_fc.T[128:]], axis=1)  # [128, 12]
    shared = {
        "wiha": np.ascontiguousarray(wiha).astype(np.float32),
        "whh0": np.ascontiguousarray(whh_t[:128]).astype(np.float32),
        "whh1": np.ascontiguousarray(whh_t[128:]).astype(np.float32),
        "wfc": np.ascontiguousarray(wfc_fold).astype(np.float32),
        "bfc": np.ascontiguousarray(b_fc.reshape(O, 1)).astype(np.float32),
    }
    ones = np.ones((S, 1, BL), dtype=np.float32)
    in_maps = []
    for c in range(NCORES):
        xc = x[c * BL : (c + 1) * BL]  # [BL, S, F]
        xT = np.ascontiguousarray(xc.transpose(1, 2, 0)).astype(np.float32)
        xaug = np.concatenate([xT, ones], axis=1)  # [S, 33, BL]
        in_maps.append({"xaug": np.ascontiguousarray(xaug), **shared})
    return in_maps


_NC_CACHE = {}


def _get_nc():
    if "nc" not in _NC_CACHE:
        _NC_CACHE["nc"] = build_nc()
    return _NC_CACHE["nc"]


def run(inputs, trace=False):
    in_maps = _host_prep(**inputs)
    nc = _get_nc()
    res = run_bass_kernel_spmd(nc, in_maps, core_ids=list(range(NCORES)), trace=trace)
    outs = []
    for c in range(NCORES):
        o = res.results[c]["out_t"]  # [33, 6, BL]
        outs.append(np.ascontiguousarray(o.transpose(2, 0, 1)))  # [BL, 33, 6]
    full = np.concatenate(outs, axis=0).astype(np.float32)  # [B, 33, 6]
    return full, res


def kernel(**inputs):
    full, _ = run(inputs, trace=False)
    return full


if __name__ == "__main__":
    t = build_nc()
    print("built ok")
